# revision 12
# baseline (speedup 1.0000x reference)
"""Trainium2 Bass kernel for nn_Attention_9981503996487.

Single-layer attention prefill (B=1, S=4096, H=2048, 16 q-heads, 4 kv-heads,
D=128, RoPE, causal, GQA, empty KV cache at cache_position=0).

Sharding (tensor parallel over heads, per the hint): core c owns q-heads
{2c, 2c+1} and kv-head c//2.  wq/wk/wv are split column-wise, wo row-wise;
each core computes a partial o_proj output over its 256 head-channels and
the host sums the 8 partials (the "all-reduce").

Per-core device program (matmul datapath in fp16, fp32 PSUM):
  A) QKV projection: hiddenT streamed in 512-column chunks; a packed
     [2048, 512] weight block (q0|q1|k|v) accumulates 4 PSUM tiles over 16
     contraction tiles, producing qT/kT/vT in [d=128, s] layout.  RoPE is
     applied in that layout via a DMA half-swap plus sign-folded sin.  V is
     transposed to [s, d] tiles with PE transposes (fp16, 1 cyc/row).
     Weight/hidden DMAs are interleaved so the first matmul starts ~1us in.
  B) Flash-style causal attention per 512-query chunk with the two heads'
     pipelines interleaved (gives the ACT engine a full round of slack):
     scoresT tile [k=128, q<=512] = kT_tile^T @ qT_chunk; exp on ACT
     (PSUM->SBUF fp16) with the 1/sqrt(D) scale; multiplicative causal mask
     on diagonal tiles; O^T accumulation via V_tile^T @ P; softmax
     denominators accumulated in fp16 on DVE (4x mode) and broadcast via an
     fp16 ones matmul (1 cyc/row); fast-approx reciprocal; one DVE multiply.
  C) o_proj: out[s_tile, hid_chunk] accumulated over the two heads' OT
     slices; result DMA'd to DRAM as the core's partial fp32 output.
"""

import math
import os

import numpy as np

S = 4096
HID = 2048
D = 128
NCORES = 8
CH = 512          # query / s-chunk width
NCH = S // CH     # 8 chunks
NT = HID // 128   # 16 contraction tiles
SCALE = 1.0 / math.sqrt(D)
DT_NAME = os.environ.get("BASSK_DTYPE", "fp16")


def _build_nc():
    import concourse.bacc as bacc
    import concourse.mybir as mybir
    import concourse.tile as tile

    f32 = mybir.dt.float32
    if DT_NAME == "fp16":
        DT = mybir.dt.float16
    elif DT_NAME == "bf16":
        DT = mybir.dt.bfloat16
    else:
        DT = mybir.dt.float32r
    # dtype for the V-transpose path (PE transpose requires out == lhsT dtype)
    TDT = DT if DT_NAME in ("fp16", "bf16") else mybir.dt.float32r
    EXP = mybir.ActivationFunctionType.Exp

    nc = bacc.Bacc("TRN2", target_bir_lowering=False, debug=False)

    hT = nc.dram_tensor("hT", [HID, S], DT, kind="ExternalInput")
    wcat = nc.dram_tensor("wcat", [HID, 384], DT, kind="ExternalInput")
    wo2 = nc.dram_tensor("wo2", [256, HID], DT, kind="ExternalInput")
    cosT = nc.dram_tensor("cosT", [128, S], DT, kind="ExternalInput")
    sinTs = nc.dram_tensor("sinTs", [128, S], DT, kind="ExternalInput")
    mask4 = nc.dram_tensor("mask4", [128, 4 * CH], DT, kind="ExternalInput")
    ident = nc.dram_tensor("ident", [128, 128], TDT, kind="ExternalInput")
    onesw = nc.dram_tensor("onesw", [128, 128], DT, kind="ExternalInput")
    out = nc.dram_tensor("out", [S, HID], DT, kind="ExternalOutput")
    # pairwise k/v exchange scratch: each core computes only its own raw
    # k (even cores) or v (odd cores); the AllGather output is uniformly
    # [k_raw | v_raw] on both cores of a pair
    kvout = nc.dram_tensor("kvout", [NCH, 128, CH], DT, kind="Internal")
    kvag = nc.dram_tensor("kvag", [NCH, 256, CH], DT, kind="Internal")
    CC_GROUPS = [[0, 1], [2, 3], [4, 5], [6, 7]]

    with tile.TileContext(nc) as tc:
        with tc.tile_pool(name="persist", bufs=1) as persist:
            qt0 = persist.tile([128, S], DT, name="qt0")
            qt1 = persist.tile([128, S], DT, name="qt1")
            ktt = persist.tile([128, S], DT, name="ktt")
            vsb = persist.tile([128, S], DT, name="vsb")
            id_sb = persist.tile([128, 128], TDT, name="id_sb")
            ones_sb = persist.tile([128, 128], DT, name="ones_sb")
            qdest = [qt0, qt1, ktt]

            # ---------------- Stage A: QKV projection + RoPE ----------------
            with (
                tc.tile_pool(name="aw", bufs=1) as aw,
                tc.tile_pool(name="ah", bufs=2) as ah,
                tc.tile_pool(name="ax", bufs=4) as ax,
                tc.tile_pool(name="psA", bufs=2, space="PSUM") as psA,
                tc.tile_pool(name="psTR", bufs=2, space="PSUM") as psTR,
            ):
                # weight slices on the sync queue, chunk-0 hidden slices on
                # the gpsimd queue: the first accumulation starts after one
                # DMA on each queue
                wcat_sb = aw.tile([128, NT * 384], DT, name="wcat_sb")
                htile0 = ah.tile([128, NT * CH], DT, name="htile", tag="htile")
                for t in range(NT):
                    nc.sync.dma_start(
                        wcat_sb[:, t * 384:(t + 1) * 384],
                        wcat[t * 128:(t + 1) * 128, :],
                    )
                    nc.gpsimd.dma_start(
                        htile0[:, t * CH:(t + 1) * CH],
                        hT[t * 128:(t + 1) * 128, 0:CH],
                    )

                def rope(dest, src, s0, cos_c, sin_c):
                    # RoPE: dest = x * cosT + halfswap(x) * signed_sinT
                    swap = ax.tile([128, CH], DT, name="swap", tag="swap")
                    nc.gpsimd.dma_start(swap[0:64, :], src[64:128, :])
                    nc.gpsimd.dma_start(swap[64:128, :], src[0:64, :])
                    t1 = ax.tile([128, CH], DT, name="t1", tag="t1")
                    nc.vector.tensor_mul(t1[:], src[:], cos_c[:])
                    t2 = ax.tile([128, CH], DT, name="t2", tag="t2")
                    nc.vector.tensor_mul(t2[:], swap[:], sin_c[:])
                    nc.vector.tensor_add(dest[:], t1[:], t2[:])

                def handle_out(o, ps, ci, cos_c, sin_c):
                    s0 = ci * CH
                    if o < 2:
                        # q0/q1: evacuate then RoPE in-place into qt
                        x_sb = ax.tile([128, CH], DT, name="x_sb", tag="evac")
                        nc.vector.tensor_copy(x_sb[:], ps[:])
                        rope(qdest[o][:, s0:s0 + CH], x_sb, s0, cos_c, sin_c)
                    else:
                        # raw k (even cores) / raw v (odd cores): evacuate
                        # and exchange with the pair peer; the gathered
                        # buffer is [k_raw | v_raw] on both cores
                        xout = ax.tile([128, CH], DT, name="xout", tag="xout")
                        nc.scalar.copy(xout[:], ps[:])
                        nc.gpsimd.dma_start(kvout[ci], xout[:])
                        nc.gpsimd.collective_compute(
                            "AllGather",
                            mybir.AluOpType.bypass,
                            replica_groups=CC_GROUPS,
                            ins=[kvout[ci].opt()],
                            outs=[kvag[ci].opt()],
                        )
                        kraw = ax.tile([128, CH], DT, name="kraw", tag="kraw")
                        nc.gpsimd.dma_start(kraw[:], kvag[ci, 0:128, :])
                        vraw = ax.tile([128, CH], DT, name="vraw", tag="vraw")
                        nc.gpsimd.dma_start(vraw[:], kvag[ci, 128:256, :])
                        rope(ktt[:, s0:s0 + CH], kraw, s0, cos_c, sin_c)
                        # V: transpose [d, s] -> [s, d] blocks
                        trp = psTR.tile([128, CH], TDT, name="trp", tag="trp")
                        for b in range(4):
                            nc.tensor.transpose(
                                trp[:, b * 128:(b + 1) * 128],
                                vraw[:, b * 128:(b + 1) * 128],
                                id_sb[:],
                            )
                        nc.scalar.copy(vsb[:, s0:s0 + CH], trp[:])

                # chunk 0: t-outer loop over 3 parallel accumulators so the
                # PE consumes each hidden slice as soon as its DMA lands
                cos_c = ah.tile([128, CH], DT, name="cos_c", tag="cos_c")
                nc.gpsimd.dma_start(cos_c[:], cosT[:, 0:CH])
                sin_c = ah.tile([128, CH], DT, name="sin_c", tag="sin_c")
                nc.gpsimd.dma_start(sin_c[:], sinTs[:, 0:CH])
                nc.sync.dma_start(id_sb[:], ident[:])
                nc.sync.dma_start(ones_sb[:], onesw[:])
                ps0 = [
                    psA.tile([128, CH], f32, name=f"ps0_{o}", tag=f"ps0_{o}",
                             bufs=1)
                    for o in range(3)
                ]
                for t in range(NT):
                    for o in range(3):
                        wsl = wcat_sb[:, t * 384 + o * 128:t * 384 + (o + 1) * 128]
                        nc.tensor.matmul(
                            ps0[o][:], wsl, htile0[:, t * CH:(t + 1) * CH],
                            start=(t == 0), stop=(t == NT - 1),
                        )
                for o in range(3):
                    handle_out(o, ps0[o], 0, cos_c, sin_c)

                for ci in range(1, NCH):
                    s0 = ci * CH
                    htile = ah.tile([128, NT * CH], DT, name="htile",
                                    tag="htile")
                    for t in range(NT):
                        nc.sync.dma_start(
                            htile[:, t * CH:(t + 1) * CH],
                            hT[t * 128:(t + 1) * 128, s0:s0 + CH],
                        )
                    cos_c = ah.tile([128, CH], DT, name="cos_c", tag="cos_c")
                    nc.gpsimd.dma_start(cos_c[:], cosT[:, s0:s0 + CH])
                    sin_c = ah.tile([128, CH], DT, name="sin_c", tag="sin_c")
                    nc.gpsimd.dma_start(sin_c[:], sinTs[:, s0:s0 + CH])

                    for o in range(3):
                        ps = psA.tile([128, CH], f32, name="psA_t", tag="psA_t")
                        for t in range(NT):
                            wsl = wcat_sb[:, t * 384 + o * 128:t * 384 + (o + 1) * 128]
                            nc.tensor.matmul(
                                ps[:], wsl, htile[:, t * CH:(t + 1) * CH],
                                start=(t == 0), stop=(t == NT - 1),
                            )
                        handle_out(o, ps, ci, cos_c, sin_c)

            # ---------------- Stage B + C: attention and o_proj ----------------
            with (
                tc.tile_pool(name="bw", bufs=1) as bw,
                tc.tile_pool(name="bp", bufs=8) as bp,
                tc.tile_pool(name="bo", bufs=4) as bo,
                tc.tile_pool(name="br", bufs=4) as brp,
                tc.tile_pool(name="bd", bufs=4) as bd,
                tc.tile_pool(name="co", bufs=6) as co,
                tc.tile_pool(name="psST", bufs=2, space="PSUM") as psST,
                tc.tile_pool(name="psOT", bufs=2, space="PSUM") as psOT,
                tc.tile_pool(name="psO", bufs=2, space="PSUM") as psO,
            ):
                mask_sb = bw.tile([128, 4 * CH], DT, name="mask_sb")
                nc.sync.dma_start(mask_sb[:], mask4[:])
                wo_sb0 = bw.tile([128, HID], DT, name="wo_sb0")
                nc.sync.dma_start(wo_sb0[:], wo2[0:128, :])
                wo_sb1 = bw.tile([128, HID], DT, name="wo_sb1")
                nc.sync.dma_start(wo_sb1[:], wo2[128:256, :])

                for ci in range(NCH):
                    s0 = ci * CH
                    n_kt = 4 * (ci + 1)
                    n_full_pairs = (4 * ci) // 2

                    # per-head state: OT psum accumulator + fp16 den halves
                    qts = [qt0, qt1]
                    ot_ps = [
                        psOT.tile([128, CH], f32, name="ot_ps", tag=f"ot{h}",
                                  bufs=1)
                        for h in range(2)
                    ]
                    den = [
                        bd.tile([128, 2 * CH], DT, name="den", tag=f"den{h}")
                        for h in range(2)
                    ]
                    if n_full_pairs == 0:
                        for h in range(2):
                            nc.vector.memset(den[h][:], 0.0)

                    def emit_pair(h, j):
                        # two non-diagonal k-tiles share one 2-bank psum
                        # tile and a single exp
                        k0 = 2 * j
                        stp = psST.tile([128, 2 * CH], f32, name="st_ps",
                                        tag="st")
                        for half in range(2):
                            kt = k0 + half
                            nc.tensor.matmul(
                                stp[:, half * CH:(half + 1) * CH],
                                ktt[:, kt * 128:(kt + 1) * 128],
                                qts[h][:, s0:s0 + CH],
                                start=True, stop=True,
                            )
                        p_sb = bp.tile([128, 2 * CH], DT, name="p_sb", tag="p")
                        nc.scalar.activation(p_sb[:], stp[:], EXP, scale=SCALE)
                        return p_sb

                    def consume_pair(h, j, p_sb):
                        k0 = 2 * j
                        for half in range(2):
                            kt = k0 + half
                            nc.tensor.matmul(
                                ot_ps[h][:],
                                vsb[:, kt * 128:(kt + 1) * 128],
                                p_sb[:, half * CH:(half + 1) * CH],
                                start=(kt == 0), stop=(kt == n_kt - 1),
                            )
                        if j == 0:
                            nc.vector.tensor_copy(den[h][:], p_sb[:])
                        else:
                            nc.vector.tensor_add(den[h][:], den[h][:], p_sb[:])

                    def emit_diag(h, kt):
                        ridx = kt - 4 * ci
                        off = ridx * 128
                        stp = psST.tile([128, 2 * CH], f32, name="st_ps",
                                        tag="st")
                        nc.tensor.matmul(
                            stp[:, off:CH],
                            ktt[:, kt * 128:(kt + 1) * 128],
                            qts[h][:, s0 + off:s0 + CH],
                            start=True, stop=True,
                        )
                        p_sb = bp.tile([128, 2 * CH], DT, name="p_sb", tag="p")
                        nc.scalar.activation(
                            p_sb[:, off:CH], stp[:, off:CH], EXP, scale=SCALE
                        )
                        nc.vector.tensor_mul(
                            p_sb[:, off:off + 128], p_sb[:, off:off + 128],
                            mask_sb[:, ridx * CH + off:ridx * CH + off + 128],
                        )
                        return p_sb

                    def consume_diag(h, kt, p_sb):
                        ridx = kt - 4 * ci
                        off = ridx * 128
                        nc.tensor.matmul(
                            ot_ps[h][:, off:CH],
                            vsb[:, kt * 128:(kt + 1) * 128],
                            p_sb[:, off:CH],
                            start=(kt == 0), stop=(kt == n_kt - 1),
                        )
                        side = (kt % 2) * CH
                        nc.vector.tensor_add(
                            den[h][:, side + off:side + CH],
                            den[h][:, side + off:side + CH],
                            p_sb[:, off:CH],
                        )

                    def emit(h, item):
                        kind, idx = item
                        return emit_pair(h, idx) if kind == "pair" else emit_diag(h, idx)

                    def consume(h, item, p):
                        kind, idx = item
                        if kind == "pair":
                            consume_pair(h, idx, p)
                        else:
                            consume_diag(h, idx, p)

                    # interleave the two heads' emit/consume streams with a
                    # 1-item skew: PE alternates scores(h0), scores(h1),
                    # PV(h0), PV(h1) while ACT exps the previous round
                    def normalize(h):
                        # merge den halves (fp16 DVE 4x), broadcast with one
                        # fp16 ones matmul, fast approx reciprocal, then
                        # normalize the OT accumulator on eviction
                        nc.vector.tensor_add(
                            den[h][:, 0:CH], den[h][:, 0:CH],
                            den[h][:, CH:2 * CH],
                        )
                        bc_ps = psO.tile([128, CH], f32, name="bc_ps", tag="ops")
                        nc.tensor.matmul(
                            bc_ps[:], ones_sb[:], den[h][:, 0:CH],
                            start=True, stop=True,
                        )
                        recip = brp.tile([128, CH], f32, name="recip",
                                         tag="recip")
                        nc.vector.reciprocal_approx_fast(recip[:], bc_ps[:])
                        ot_sb = bo.tile([128, CH], DT, name="ot_sb", tag=f"ot{h}")
                        nc.vector.tensor_mul(ot_sb[:], ot_ps[h][:], recip[:])
                        return ot_sb

                    work = [("pair", j) for j in range(n_full_pairs)]
                    work += [("diag", 4 * ci + r) for r in range(4)]
                    prev = None
                    for item in work:
                        cur0 = emit(0, item)
                        cur1 = emit(1, item)
                        if prev is not None:
                            pitem, pp0, pp1 = prev
                            consume(0, pitem, pp0)
                            consume(1, pitem, pp1)
                        prev = (item, cur0, cur1)
                    pitem, pp0, pp1 = prev
                    consume(0, pitem, pp0)
                    ot_tiles = [normalize(0)]
                    consume(1, pitem, pp1)
                    ot_tiles.append(normalize(1))

                    # Stage C: o_proj, direct PSUM -> DRAM stores
                    for st_i in range(4):
                        row = (ci * 4 + st_i) * 128
                        for hc in range(4):
                            ops = psO.tile([128, 512], f32, name="ops", tag="ops")
                            nc.tensor.matmul(
                                ops[:],
                                ot_tiles[0][:, st_i * 128:(st_i + 1) * 128],
                                wo_sb0[:, hc * 512:(hc + 1) * 512],
                                start=True, stop=False,
                            )
                            nc.tensor.matmul(
                                ops[:],
                                ot_tiles[1][:, st_i * 128:(st_i + 1) * 128],
                                wo_sb1[:, hc * 512:(hc + 1) * 512],
                                start=False, stop=True,
                            )
                            o_sb = co.tile([128, 512], DT, name="o_sb", tag="o_sb")
                            if (st_i * 4 + hc) % 4 == 0:
                                nc.scalar.copy(o_sb[:], ops[:])
                            else:
                                nc.vector.tensor_copy(o_sb[:], ops[:])
                            nc.sync.dma_start(
                                out[row:row + 128, hc * 512:(hc + 1) * 512],
                                o_sb[:],
                            )

    nc.finalize()
    return nc


def _host_prep(hidden_states, cos, sin, position_ids, wq, wk, wv, wo):
    """Build the 8 per-core input maps."""
    if DT_NAME == "fp16":
        np_dt = np.float16
    elif DT_NAME == "bf16":
        import ml_dtypes
        np_dt = ml_dtypes.bfloat16
    else:
        np_dt = np.float32
    tdt = np_dt if DT_NAME in ("fp16", "bf16") else np.float32

    hidden = np.asarray(hidden_states, dtype=np.float32)[0]        # [S, HID]
    hT = np.ascontiguousarray(hidden.T).astype(np_dt)              # [HID, S]
    pos = np.asarray(position_ids)[0].astype(np.int64)             # [S]
    cos_np = np.asarray(cos, dtype=np.float32)[pos]                # [S, 64]
    sin_np = np.asarray(sin, dtype=np.float32)[pos]
    cos_full = np.concatenate([cos_np, cos_np], axis=1)            # [S, 128]
    sin_full = np.concatenate([sin_np, sin_np], axis=1)
    cosT = np.ascontiguousarray(cos_full.T).astype(np_dt)          # [128, S]
    sinTs = np.ascontiguousarray(sin_full.T)
    sinTs[0:64, :] *= -1.0                                         # sign fold
    sinTs = sinTs.astype(np_dt)

    # multiplicative causal masks for the 4 diagonal tile offsets
    kk = np.arange(128)[:, None]
    jj = np.arange(CH)[None, :]
    mask4 = np.concatenate(
        [(kk + ridx * 128 <= jj).astype(np_dt) for ridx in range(4)], axis=1
    )                                                              # [128, 2048]
    ident = np.eye(128, dtype=tdt)
    onesw = np.ones((128, 128), dtype=np_dt)

    wq_np = np.asarray(wq, dtype=np.float32)
    wk_np = np.asarray(wk, dtype=np.float32)
    wv_np = np.asarray(wv, dtype=np.float32)
    wo_np = np.asarray(wo, dtype=np.float32)

    in_maps = []
    for c in range(NCORES):
        h0 = 2 * c
        g = c // 2
        # even core of a pair computes raw k, odd core raw v
        wx = wk_np if c % 2 == 0 else wv_np
        wcat = np.ascontiguousarray(np.concatenate(
            [
                wq_np[:, h0 * D:(h0 + 1) * D],
                wq_np[:, (h0 + 1) * D:(h0 + 2) * D],
                wx[:, g * D:(g + 1) * D],
            ],
            axis=1,
        )).astype(np_dt)                                           # [HID, 384]
        wo2 = np.ascontiguousarray(
            wo_np[h0 * D:(h0 + 2) * D, :]
        ).astype(np_dt)                                            # [256, HID]
        in_maps.append({
            "hT": hT,
            "wcat": wcat,
            "wo2": wo2,
            "cosT": cosT,
            "sinTs": sinTs,
            "mask4": mask4,
            "ident": ident,
            "onesw": onesw,
        })
    return in_maps


_NC_CACHE = [None]


def _run(inputs, trace=False, tmpdir=None):
    from concourse import bass_utils

    in_maps = _host_prep(
        inputs["hidden_states"], inputs["cos"], inputs["sin"],
        inputs["position_ids"], inputs["wq"], inputs["wk"], inputs["wv"],
        inputs["wo"],
    )
    if _NC_CACHE[0] is None:
        _NC_CACHE[0] = _build_nc()
    nc = _NC_CACHE[0]
    res = bass_utils.run_bass_kernel_spmd(
        nc, in_maps, core_ids=list(range(NCORES)), trace=trace, tmpdir=tmpdir,
    )
    acc = res.results[0]["out"].astype(np.float32)
    for c in range(1, NCORES):
        acc = acc + res.results[c]["out"].astype(np.float32)
    return acc.reshape(1, S, HID), res


def kernel(**inputs):
    out, _ = _run(inputs, trace=False)
    return out
